# revision 1
# baseline (speedup 1.0000x reference)
"""Trainium2 Bass kernel for nn_BlockWiseSemanticAttention_944892805640.

Sharding: 8 cores = (batch b = c//2) x (T-half h = c%2). Each core computes
its 512 query rows of one batch element end-to-end; keys/values span the
full T=1024. Two tiny pairwise collectives: AllGather of the block-attention
output (cross-attn K/V needs full T) and ReduceScatter of the attention-
received partial sums.

Device algorithm (validated in numpy proto against the jax reference):
  blocks = to_blocks(M)  (host, pure permutation)
  per (blk,head) pair p of 32: S.T = k_p^T q_p (PE, f32r), U = exp(S/sqrt2)
  (ACT), [o0,o1,r] = [v_p|1]^T U (PE accum).  Normalize token-major, apply
  block-diag Wo_blk, AllGather -> cross attention (8 heads, same scheme,
  plus attn-received partials via DVE mult+reduce), Wo_c, LN1, FFN (gelu),
  LN2, sens = sigmoid(emb[token] + recv*alpha), out = x + (y-x)*sens.

f32r (TF32-like, 1 cyc/row) is used for all T^2 matmuls; fp32 elsewhere.
Workarounds for this toolchain (measured on HW):
  - matmul operand base partitions must be 0/32/64/96 -> Q/K tiles place
    each pair at a 32-aligned partition slot (free: it is just lhsT column
    ordering in the QKV projection matmuls).
  - walrus accepts at most ONE sync-wait per instruction -> post-pass splits
    extra waits onto same-engine NoOps.
  - SWDGE queue >0 semaphores never fire under this runtime ->
    num_swdge_queues=1 and all DMA on gpsimd.
  - f32r operands must be produced by rounding compute ops (DVE/ACT writes).
"""
import sys

sys.path.insert(0, '/opt/trn_rl_repo')

import numpy as np

import concourse.bass as bass
import concourse.tile as tile
from concourse import mybir
from concourse.bass_utils import run_bass_kernel_spmd

AF = mybir.ActivationFunctionType
ALU = mybir.AluOpType
AX = mybir.AxisListType
f32 = mybir.dt.float32
f32r = mybir.dt.float32r
i32 = mybir.dt.int32

B, T, TQ, P = 4, 1024, 512, 128
NBLK = 16
LN_EPS = 1e-5
INV_SQRT2 = 0.7071067811865476
INV_SQRT8 = 0.35355339059327373
REPLICA_GROUPS = [[0, 1], [2, 3], [4, 5], [6, 7]]


def _legalize_waits(nc):
    """This walrus build accepts at most ONE sync-wait per instruction.
    Split extras onto same-engine NoOps placed immediately before (engine
    streams execute in block order, so this is safe)."""
    n_split = 0
    for func in nc.m.functions:
        for bb in func.blocks:
            out, changed = [], False
            for inst in bb.instructions:
                si = inst.sync_info
                if si is not None and si.on_wait is not None and len(si.on_wait) > 1:
                    waits = list(si.on_wait)
                    for i, w in enumerate(waits[:-1]):
                        nop = mybir.InstNoOp(name=f"{inst.name}-lw{i}", ins=[], outs=[])
                        nop.engine = inst.engine
                        nop.sync_info = mybir.SyncInfo(on_wait=[w], on_update=[])
                        out.append(nop)
                    inst.sync_info = mybir.SyncInfo(
                        on_wait=[waits[-1]], on_update=list(si.on_update or []))
                    changed = True
                    n_split += 1
                upd = inst.sync_info.on_update if inst.sync_info else None
                assert not upd or len(upd) <= 1, f"multi-update on {inst.name}"
                out.append(inst)
            if changed:
                bb.instructions = out
    return n_split


def build_program():
    nc = bass.Bass(num_swdge_queues=1)

    # ---- I/O ----
    xt_aug_d = nc.dram_tensor("xt_aug", [65, T], f32, kind="ExternalInput")
    xtq_aug_d = nc.dram_tensor("xtq_aug", [65, TQ], f32, kind="ExternalInput")
    x_tok_d = nc.dram_tensor("x_tok", [TQ, 64], f32, kind="ExternalInput")
    wq_d = nc.dram_tensor("wq", [8, 65, 128], f32, kind="ExternalInput")
    wk_d = nc.dram_tensor("wk", [8, 65, 128], f32, kind="ExternalInput")
    wv96_d = nc.dram_tensor("wv96", [65, 96], f32, kind="ExternalInput")
    wcq_d = nc.dram_tensor("wcq", [2, 65, 128], f32, kind="ExternalInput")
    wck_d = nc.dram_tensor("wck", [2, 65, 128], f32, kind="ExternalInput")
    wcv72_d = nc.dram_tensor("wcv72", [65, 72], f32, kind="ExternalInput")
    wobig_d = nc.dram_tensor("wobig", [65, 64], f32, kind="ExternalInput")
    woc_d = nc.dram_tensor("woc", [65, 64], f32, kind="ExternalInput")
    w1_d = nc.dram_tensor("w1", [65, 256], f32, kind="ExternalInput")
    w2_d = nc.dram_tensor("w2", [2, 128, 64], f32, kind="ExternalInput")
    b2_d = nc.dram_tensor("b2", [64, 1], f32, kind="ExternalInput")
    lnp_d = nc.dram_tensor("lnp", [4, 64], f32, kind="ExternalInput")  # g1,be1,g2,be2
    alpha_d = nc.dram_tensor("alpha", [1, 16], f32, kind="ExternalInput")
    semb_d = nc.dram_tensor("semb", [32000, 16], f32, kind="ExternalInput")
    tokidx_d = nc.dram_tensor("tokidx", [P, 4], i32, kind="ExternalInput")
    ident_d = nc.dram_tensor("ident", [P, P], f32, kind="ExternalInput")
    e4_d = nc.dram_tensor("e4", [4, 4, P], f32, kind="ExternalInput")
    out_d = nc.dram_tensor("out", [TQ, 64], f32, kind="ExternalOutput")

    # internal DRAM for collectives
    ab_half_d = nc.dram_tensor("ab_half", [64, TQ], f32)
    ab_gath_d = nc.dram_tensor("ab_gath", [128, TQ], f32)
    recv_full_d = nc.dram_tensor("recv_full", [T], f32)
    recv_rs_d = nc.dram_tensor("recv_rs", [TQ], f32)

    with tile.TileContext(nc) as tc:
        with tc.tile_pool(name="const", bufs=1) as cst, \
             tc.tile_pool(name="work", bufs=1) as wrk, \
             tc.tile_pool(name="ut", bufs=10) as utp, \
             tc.tile_pool(name="sc2", bufs=2) as sc2, \
             tc.tile_pool(name="ps_score", bufs=2, space="PSUM") as pss, \
             tc.tile_pool(name="ps_acc", bufs=2, space="PSUM") as psa, \
             tc.tile_pool(name="ps_tok", bufs=1, space="PSUM") as pstok, \
             tc.tile_pool(name="ps_misc", bufs=1, space="PSUM") as psm:

            # ================= load constants =================
            def dmain(shape, dt, src, tag):
                t = cst.tile(shape, dt, tag=tag)
                nc.gpsimd.dma_start(t[:], src)
                return t

            xt_sb = dmain([65, T], f32, xt_aug_d[:], "xt")
            xtq_sb = dmain([65, TQ], f32, xtq_aug_d[:], "xtq")
            ident = dmain([P, P], f32, ident_d[:], "ident")
            wv96_sb = dmain([65, 96], f32, wv96_d[:], "wv96")
            wcv72_sb = dmain([65, 72], f32, wcv72_d[:], "wcv72")
            wobig_sb = dmain([65, 64], f32, wobig_d[:], "wobig")
            woc_sb = dmain([65, 64], f32, woc_d[:], "woc")
            w1_sb = dmain([65, 256], f32, w1_d[:], "w1")
            w2_sb = [dmain([P, 64], f32, w2_d[m], f"w2_{m}") for m in range(2)]
            b2_sb = dmain([64, 1], f32, b2_d[:], "b2")
            alpha_sb = dmain([1, 16], f32, alpha_d[:], "alpha")
            tok_sb_idx = dmain([P, 4], i32, tokidx_d[:], "tokidx")
            x_tok = [dmain([P, 64], f32, x_tok_d[c * P:(c + 1) * P, :], f"xtok{c}")
                     for c in range(4)]
            wq_sb = [dmain([65, 128], f32, wq_d[j], f"wq{j}") for j in range(8)]
            wk_sb = [dmain([65, 128], f32, wk_d[j], f"wk{j}") for j in range(8)]
            wcq_sb = [dmain([65, 128], f32, wcq_d[j], f"wcq{j}") for j in range(2)]
            e4_sb = [dmain([4, P], f32, e4_d[t], f"e4_{t}") for t in range(4)]
            wck_sb = [dmain([65, 128], f32, wck_d[j], f"wck{j}") for j in range(2)]
            # LN params broadcast to [P, 64]
            lnb = []
            for i in range(4):
                t = cst.tile([P, 64], f32, tag=f"lnb{i}")
                nc.gpsimd.dma_start(t[:], lnp_d[i:i + 1, :].broadcast_to([P, 64]))
                lnb.append(t)
            g1b, be1b, g2b, be2b = lnb
            # bo_blk broadcast (row 64 of wobig)
            bob = cst.tile([P, 64], f32, tag="bob")
            nc.gpsimd.dma_start(bob[:], wobig_d[64:65, :].broadcast_to([P, 64]))

            eps_p1 = cst.tile([P, 1], f32, tag="eps")
            nc.vector.memset(eps_p1[:], LN_EPS)
            ones1 = cst.tile([1, P], f32, tag="ones1")
            nc.vector.memset(ones1[:], 1.0)

            # f32r roundings of weights/inputs feeding f32r matmuls
            def to_r(src, shape, tag):
                t = cst.tile(shape, f32r, tag=tag)
                nc.vector.tensor_copy(t[:], src[:])
                return t

            xt_r = to_r(xt_sb, [65, T], "xt_r")
            xtq_r = to_r(xtq_sb, [65, TQ], "xtq_r")
            wq_r = [to_r(wq_sb[j], [65, 128], f"wq_r{j}") for j in range(8)]
            wk_r = [to_r(wk_sb[j], [65, 128], f"wk_r{j}") for j in range(8)]
            wv96_r = to_r(wv96_sb, [65, 96], "wv96_r")
            wcq_r = [to_r(wcq_sb[j], [65, 128], f"wcq_r{j}") for j in range(2)]
            wck_r = [to_r(wck_sb[j], [65, 128], f"wck_r{j}") for j in range(2)]
            wcv72_r = to_r(wcv72_sb, [65, 72], "wcv72_r")

            # ================= phase 1: block QKV =================
            # Q tiles: pair p=4j+s at partitions [32s, 32s+2)
            QT = [wrk.tile([P, TQ], f32r, tag=f"QT{j}", name=f"QT{j}")
                  for j in range(8)]
            for jq in range(4):
                q_ps = pss.tile([P, 2 * TQ], f32, tag="s", name=f"qps{jq}")
                for c in range(2):
                    nc.tensor.matmul(q_ps[:, c * TQ:(c + 1) * TQ],
                                     wq_r[2 * jq + c][:], xtq_r[:],
                                     start=True, stop=True)
                    nc.vector.tensor_copy(QT[2 * jq + c][:],
                                          q_ps[:, c * TQ:(c + 1) * TQ])
            KT = []
            for j in range(8):
                t = wrk.tile([P, T], f32r, tag=f"KT{j}")
                k_ps = pss.tile([P, 2 * TQ], f32, tag="s", name=f"kps{j}")
                for c in range(2):
                    nc.tensor.matmul(k_ps[:, c * TQ:(c + 1) * TQ], wk_r[j][:],
                                     xt_r[:, c * TQ:(c + 1) * TQ],
                                     start=True, stop=True)
                nc.vector.tensor_copy(t[:], k_ps[:])
                KT.append(t)
            VA = [wrk.tile([P, 96], f32r, tag=f"VA{kc}", name=f"VA{kc}")
                  for kc in range(8)]
            for kv in range(4):
                v_ps = pss.tile([P, 2 * TQ], f32, tag="s", name=f"vps{kv}")
                for c in range(2):
                    kc = 2 * kv + c
                    nc.tensor.matmul(v_ps[:, c * TQ:c * TQ + 96],
                                     xt_r[:, kc * P:(kc + 1) * P], wv96_r[:],
                                     start=True, stop=True)
                    nc.vector.tensor_copy(VA[kc][:], v_ps[:, c * TQ:c * TQ + 96])

            # ================= phase 2: block attention =================
            # token-major accumulator of transposed AV results:
            # col 96*t4 + 3*p + {0,1,2} = (o0, o1, r) of pair p, token block t4
            tok_ps = pstok.tile([P, 384], f32, tag="tok", name="tok_ps")
            for j in range(8):
                for sp_i in range(2):          # two pairs per score tile
                    sa, sb_ = 2 * sp_i, 2 * sp_i + 1
                    pa, pb = 4 * j + sa, 4 * j + sb_
                    av_a = psa.tile([3, TQ], f32, tag="av")
                    av_b = psa.tile([3, TQ], f32, tag="av")
                    for kc in range(8):
                        s_ps = pss.tile([P, 2 * TQ], f32, tag="s")
                        nc.tensor.matmul(s_ps[:, 0:TQ],
                                         KT[j][32 * sa:32 * sa + 2, kc * P:(kc + 1) * P],
                                         QT[j][32 * sa:32 * sa + 2, :],
                                         start=True, stop=True,
                                         tile_position=(32 * sa, 0))
                        nc.tensor.matmul(s_ps[:, TQ:2 * TQ],
                                         KT[j][32 * sb_:32 * sb_ + 2, kc * P:(kc + 1) * P],
                                         QT[j][32 * sb_:32 * sb_ + 2, :],
                                         start=True, stop=True,
                                         tile_position=(32 * sb_, 0))
                        ut = utp.tile([P, 2 * TQ], f32r, tag="ut")
                        nc.scalar.activation(ut[:], s_ps[:], AF.Exp, scale=INV_SQRT2)
                        nc.tensor.matmul(av_a[:], VA[kc][:, 3 * pa:3 * pa + 3],
                                         ut[:, 0:TQ],
                                         start=(kc == 0), stop=(kc == 7))
                        nc.tensor.matmul(av_b[:], VA[kc][:, 3 * pb:3 * pb + 3],
                                         ut[:, TQ:2 * TQ],
                                         start=(kc == 0), stop=(kc == 7))
                    for pp, av in ((pa, av_a), (pb, av_b)):
                        avs = sc2.tile([3, TQ], f32, tag="avsb", name=f"avs{pp}")
                        nc.vector.tensor_copy(avs[:], av[:])
                        for t4 in range(4):
                            nc.tensor.matmul(
                                tok_ps[:, 96 * t4 + 3 * pp:96 * t4 + 3 * pp + 3],
                                avs[:, t4 * P:(t4 + 1) * P], ident[0:3, 0:3],
                                is_transpose=True, start=True, stop=True)

            # ================= phase 3: normalize + project + gather =================
            abT_aug = wrk.tile([65, TQ], f32, tag="abT")
            nc.vector.memset(abT_aug[64:65, :], 1.0)
            for tc4 in range(4):
                tok96 = wrk.tile([P, 96], f32, tag="tok96")
                nc.vector.tensor_copy(tok96[:], tok_ps[:, 96 * tc4:96 * tc4 + 96])
                nc.vector.reciprocal(tok96[:, 2:96:3], tok96[:, 2:96:3])
                abpre = wrk.tile([P, 64], f32, tag="abpre")
                g3 = tok96[:].rearrange("p (g c) -> p g c", c=3)
                nc.vector.tensor_tensor(
                    abpre[:].rearrange("p (g c) -> p g c", c=2),
                    g3[:, :, 0:2], g3[:, :, 2:3].broadcast_to([P, 32, 2]), ALU.mult)
                tb_ps = psm.tile([P, TQ], f32, tag="m")
                nc.tensor.matmul(tb_ps[0:64, 0:P], abpre[:], ident[:, :],
                                 is_transpose=True, start=True, stop=True)
                nc.vector.tensor_copy(abT_aug[0:64, tc4 * P:(tc4 + 1) * P],
                                      tb_ps[0:64, 0:P])

            # feature-major projected half (for AllGather / cross K,V)
            pj_ps = psm.tile([P, TQ], f32, tag="m")
            nc.tensor.matmul(pj_ps[0:64, :], wobig_sb[:], abT_aug[:],
                             start=True, stop=True)
            abp_sb = wrk.tile([64, TQ], f32, tag="abp")
            nc.vector.tensor_copy(abp_sb[:], pj_ps[0:64, :])
            nc.gpsimd.dma_start(ab_half_d[:], abp_sb[:])
            nc.gpsimd.collective_compute(
                "AllGather", ALU.bypass, replica_groups=REPLICA_GROUPS,
                ins=[ab_half_d[:].opt()], outs=[ab_gath_d[:].opt()])

            # token-major projected (+ bo) for the residual path
            ab_tok = []
            for tc4 in range(4):
                pt_ps = psm.tile([P, TQ], f32, tag="m")
                nc.tensor.matmul(pt_ps[:, 0:64], abT_aug[0:64, tc4 * P:(tc4 + 1) * P],
                                 wobig_sb[0:64, :], start=True, stop=True)
                t = wrk.tile([P, 64], f32, tag=f"abtok{tc4}")
                nc.vector.tensor_tensor(t[:], pt_ps[:, 0:64], bob[:], ALU.add)
                ab_tok.append(t)

            # assemble full all_blocksT (+ones) and q-side (+ones), f32r copies
            abf_sb = wrk.tile([65, T], f32, tag="abf")
            nc.vector.memset(abf_sb[64:65, :], 1.0)
            nc.gpsimd.dma_start(abf_sb[0:64, 0:TQ], ab_gath_d[0:64, :])
            nc.gpsimd.dma_start(abf_sb[0:64, TQ:T], ab_gath_d[64:128, :])
            abq_sb = wrk.tile([65, TQ], f32, tag="abq")
            nc.vector.memset(abq_sb[64:65, :], 1.0)
            nc.vector.tensor_copy(abq_sb[0:64, :], abp_sb[:])
            abf_r = wrk.tile([65, T], f32r, tag="abf_r")
            nc.vector.tensor_copy(abf_r[:], abf_sb[:])
            abq_r = wrk.tile([65, TQ], f32r, tag="abq_r")
            nc.vector.tensor_copy(abq_r[:], abq_sb[:])

            # ================= phase 4: cross attention =================
            QC = [wrk.tile([P, TQ], f32r, tag=f"QC{j}", name=f"QC{j}")
                  for j in range(2)]
            qc_ps = pss.tile([P, 2 * TQ], f32, tag="s", name="qcps")
            for j in range(2):
                nc.tensor.matmul(qc_ps[:, j * TQ:(j + 1) * TQ], wcq_r[j][:],
                                 abq_r[:], start=True, stop=True)
                nc.vector.tensor_copy(QC[j][:], qc_ps[:, j * TQ:(j + 1) * TQ])
            KC = []
            for j in range(2):
                t = wrk.tile([P, T], f32r, tag=f"KC{j}")
                k_ps = pss.tile([P, 2 * TQ], f32, tag="s", name=f"kcps{j}")
                for c in range(2):
                    nc.tensor.matmul(k_ps[:, c * TQ:(c + 1) * TQ], wck_r[j][:],
                                     abf_r[:, c * TQ:(c + 1) * TQ],
                                     start=True, stop=True)
                nc.vector.tensor_copy(t[:], k_ps[:])
                KC.append(t)
            VC = [wrk.tile([P, 72], f32r, tag=f"VC{kc}", name=f"VC{kc}")
                  for kc in range(8)]
            for kv in range(4):
                v_ps = pss.tile([P, 2 * TQ], f32, tag="s", name=f"vcps{kv}")
                for c in range(2):
                    kc = 2 * kv + c
                    nc.tensor.matmul(v_ps[:, c * TQ:c * TQ + 72],
                                     abf_r[:, kc * P:(kc + 1) * P], wcv72_r[:],
                                     start=True, stop=True)
                    nc.vector.tensor_copy(VC[kc][:], v_ps[:, c * TQ:c * TQ + 72])

            # token-major accumulator: col 96*t4 + 9*h + {0..7, 8} = (vals, r)
            ctok_ps = pstok.tile([P, 384], f32, tag="tok", name="ctok_ps")
            racc = [wrk.tile([P, 8], f32, tag=f"racc{kc}", name=f"racc{kc}") for kc in range(8)]

            for jj in range(2):
                for sp_i in range(2):
                    sa, sb_ = 2 * sp_i, 2 * sp_i + 1
                    ha, hb = 4 * jj + sa, 4 * jj + sb_
                    avh = {ha: psa.tile([9, TQ], f32, tag="av", name=f"avh{ha}"),
                           hb: psa.tile([9, TQ], f32, tag="av", name=f"avh{hb}")}
                    utc = []
                    for kc in range(8):
                        s_ps = pss.tile([P, 2 * TQ], f32, tag="s")
                        nc.tensor.matmul(s_ps[:, 0:TQ],
                                         KC[jj][32 * sa:32 * sa + 8, kc * P:(kc + 1) * P],
                                         QC[jj][32 * sa:32 * sa + 8, :],
                                         start=True, stop=True,
                                         tile_position=(32 * sa, 0))
                        nc.tensor.matmul(s_ps[:, TQ:2 * TQ],
                                         KC[jj][32 * sb_:32 * sb_ + 8, kc * P:(kc + 1) * P],
                                         QC[jj][32 * sb_:32 * sb_ + 8, :],
                                         start=True, stop=True,
                                         tile_position=(32 * sb_, 0))
                        ut = utp.tile([P, 2 * TQ], f32r, tag="ut")
                        nc.scalar.activation(ut[:], s_ps[:], AF.Exp, scale=INV_SQRT8)
                        utc.append(ut)
                        for hh, off in ((ha, 0), (hb, TQ)):
                            nc.tensor.matmul(avh[hh][:],
                                             VC[kc][:, 9 * hh:9 * hh + 9],
                                             ut[:, off:off + TQ],
                                             start=(kc == 0), stop=(kc == 7))
                    for hh, off in ((ha, 0), (hb, TQ)):
                        avs = sc2.tile([9, TQ], f32, tag="avsc", name=f"avsc{hh}")
                        nc.vector.tensor_copy(avs[:], avh[hh][:])
                        rtok4 = wrk.tile([P, 4], f32, tag="rtok")
                        for t4 in range(4):
                            nc.tensor.matmul(
                                ctok_ps[:, 96 * t4 + 9 * hh:96 * t4 + 9 * hh + 9],
                                avs[:, t4 * P:(t4 + 1) * P], ident[0:9, 0:9],
                                is_transpose=True, start=True, stop=True)
                            nc.vector.reciprocal(
                                rtok4[:, t4:t4 + 1],
                                ctok_ps[:, 96 * t4 + 9 * hh + 8:96 * t4 + 9 * hh + 9])
                        # rebuild feature-major invr and broadcast across partitions
                        rt_ps = psm.tile([P, TQ], f32, tag="m")
                        nc.tensor.matmul(rt_ps[0:4, 0:P], rtok4[:], ident[:, :],
                                         is_transpose=True, start=True, stop=True)
                        rfeat = sc2.tile([4, P], f32, tag="rfeat")
                        nc.vector.tensor_copy(rfeat[:], rt_ps[0:4, 0:P])
                        ib_ps = psm.tile([P, TQ], f32, tag="m")
                        for t4 in range(4):
                            nc.tensor.matmul(ib_ps[:, t4 * P:(t4 + 1) * P],
                                             e4_sb[t4][:], rfeat[:],
                                             start=True, stop=True)
                        invrb = sc2.tile([P, TQ], f32, tag="invrb")
                        nc.vector.tensor_copy(invrb[:], ib_ps[:])
                        for kc in range(8):
                            scr = sc2.tile([P, TQ], f32, tag="scr")
                            nc.vector.tensor_tensor(
                                scr[:], utc[kc][:, off:off + TQ].bitcast(f32),
                                invrb[:], ALU.mult)
                            nc.vector.reduce_sum(racc[kc][:, hh:hh + 1], scr[:],
                                                 axis=AX.X)

            # normalize token-major, then transpose back to feature-major
            crossT_aug = wrk.tile([65, TQ], f32, tag="crossT")
            nc.vector.memset(crossT_aug[64:65, :], 1.0)
            for t4 in range(4):
                ctok96 = wrk.tile([P, 96], f32, tag="ctok96")
                nc.vector.tensor_copy(ctok96[:, 0:72], ctok_ps[:, 96 * t4:96 * t4 + 72])
                nc.vector.reciprocal(ctok96[:, 8:72:9], ctok96[:, 8:72:9])
                atok = wrk.tile([P, 64], f32, tag="atok")
                g9 = ctok96[:, 0:72].rearrange("p (g c) -> p g c", c=9)
                nc.vector.tensor_tensor(
                    atok[:].rearrange("p (g c) -> p g c", c=8),
                    g9[:, :, 0:8], g9[:, :, 8:9].broadcast_to([P, 8, 8]), ALU.mult)
                cb_ps = psm.tile([P, TQ], f32, tag="m")
                nc.tensor.matmul(cb_ps[0:64, 0:P], atok[:], ident[:, :],
                                 is_transpose=True, start=True, stop=True)
                nc.vector.tensor_copy(crossT_aug[0:64, t4 * P:(t4 + 1) * P],
                                      cb_ps[0:64, 0:P])

            # recv partials -> DRAM -> ReduceScatter -> own 512 slice
            rsum = wrk.tile([P, 8], f32, tag="rsum")
            for kc in range(8):
                nc.vector.reduce_sum(rsum[:, kc:kc + 1], racc[kc][:], axis=AX.X)
                nc.gpsimd.dma_start(recv_full_d[kc * P:(kc + 1) * P],
                                    rsum[:, kc:kc + 1])
            nc.gpsimd.collective_compute(
                "ReduceScatter", ALU.add, replica_groups=REPLICA_GROUPS,
                ins=[recv_full_d[:].opt()], outs=[recv_rs_d[:].opt()])
            recv_sb = wrk.tile([1, TQ], f32, tag="recv")
            nc.gpsimd.dma_start(recv_sb[:], recv_rs_d[None, :])
            nc.vector.tensor_scalar_mul(recv_sb[:], recv_sb[:], 1.0 / 8192.0)

            # cross output projection (feature-major)
            co_ps = psm.tile([P, TQ], f32, tag="m")
            nc.tensor.matmul(co_ps[0:64, :], woc_sb[:], crossT_aug[:],
                             start=True, stop=True)
            co_sb = wrk.tile([64, TQ], f32, tag="co")
            nc.vector.tensor_copy(co_sb[:], co_ps[0:64, :])

            # ============== phase 5/6: resid+LN1, FFN, resid+LN2 ==============
            def layernorm(x_t, gb, beb, tag, ytag):
                s1 = wrk.tile([P, 1], f32, tag=f"{tag}s1")
                nc.vector.reduce_sum(s1[:], x_t[:], axis=AX.X)
                nc.scalar.mul(s1[:], s1[:], -1.0 / 64.0)
                xc = wrk.tile([P, 64], f32, tag=f"{tag}xc")
                nc.scalar.add(xc[:], x_t[:], s1[:])
                sq = wrk.tile([P, 64], f32, tag=f"{tag}sq")
                nc.scalar.activation(sq[:], xc[:], AF.Square)
                var = wrk.tile([P, 1], f32, tag=f"{tag}v")
                nc.vector.reduce_sum(var[:], sq[:], axis=AX.X)
                nc.scalar.mul(var[:], var[:], 1.0 / 64.0)
                sd = wrk.tile([P, 1], f32, tag=f"{tag}sd")
                nc.scalar.activation(sd[:], var[:], AF.Sqrt, bias=eps_p1[:])
                rstd = wrk.tile([P, 1], f32, tag=f"{tag}rs")
                nc.vector.reciprocal(rstd[:], sd[:])
                y = wrk.tile([P, 64], f32, tag=ytag)
                nc.vector.tensor_scalar_mul(y[:], xc[:], rstd[:])
                nc.vector.tensor_tensor(y[:], y[:], gb[:], ALU.mult)
                nc.vector.tensor_tensor(y[:], y[:], beb[:], ALU.add)
                return y

            ln1_tok = []
            ln1T_aug = wrk.tile([65, TQ], f32, tag="ln1T")
            nc.vector.memset(ln1T_aug[64:65, :], 1.0)
            for tc4 in range(4):
                ct_ps = psm.tile([P, TQ], f32, tag="m")
                nc.tensor.matmul(ct_ps[:, 0:64], co_sb[:, tc4 * P:(tc4 + 1) * P],
                                 ident[0:64, 0:64], is_transpose=True,
                                 start=True, stop=True)
                x1 = wrk.tile([P, 64], f32, tag="x1")
                nc.vector.tensor_tensor(x1[:], ct_ps[:, 0:64], ab_tok[tc4][:], ALU.add)
                y = layernorm(x1, g1b, be1b, "ln1", f"ln1y{tc4}")
                ln1_tok.append(y)
                lt_ps = psm.tile([P, TQ], f32, tag="m")
                nc.tensor.matmul(lt_ps[0:64, 0:P], y[:], ident[:, :],
                                 is_transpose=True, start=True, stop=True)
                nc.vector.tensor_copy(ln1T_aug[0:64, tc4 * P:(tc4 + 1) * P],
                                      lt_ps[0:64, 0:P])

            # FFN: h = gelu(W1 @ ln1T); o2 = W2 @ h + b2
            h_sb = []
            for m in range(2):
                h_ps = psm.tile([P, TQ], f32, tag="m")
                nc.tensor.matmul(h_ps[:], w1_sb[:, m * P:(m + 1) * P], ln1T_aug[:],
                                 start=True, stop=True)
                t = wrk.tile([P, TQ], f32, tag=f"h{m}")
                nc.scalar.activation(t[:], h_ps[:], AF.Gelu)
                h_sb.append(t)
            o2_ps = psm.tile([P, TQ], f32, tag="m")
            for m in range(2):
                nc.tensor.matmul(o2_ps[0:64, :], w2_sb[m][:], h_sb[m][:],
                                 start=(m == 0), stop=(m == 1))
            o2_sb = wrk.tile([64, TQ], f32, tag="o2")
            nc.scalar.activation(o2_sb[:], o2_ps[0:64, :], AF.Identity, bias=b2_sb[:])

            y_tok = []
            for tc4 in range(4):
                ot_ps = psm.tile([P, TQ], f32, tag="m")
                nc.tensor.matmul(ot_ps[:, 0:64], o2_sb[:, tc4 * P:(tc4 + 1) * P],
                                 ident[0:64, 0:64], is_transpose=True,
                                 start=True, stop=True)
                x2 = wrk.tile([P, 64], f32, tag="x2")
                nc.vector.tensor_tensor(x2[:], ot_ps[:, 0:64], ln1_tok[tc4][:], ALU.add)
                y_tok.append(layernorm(x2, g2b, be2b, "ln2", f"ln2y{tc4}"))

            # ================= phase 7: sens + combine =================
            for tc4 in range(4):
                sa_ps = psm.tile([P, TQ], f32, tag="m")
                nc.tensor.matmul(sa_ps[:, 0:16], recv_sb[0:1, tc4 * P:(tc4 + 1) * P],
                                 alpha_sb[:], start=True, stop=True)
                emb = wrk.tile([P, 16], f32, tag="emb")
                nc.gpsimd.indirect_dma_start(
                    out=emb[:], out_offset=None, in_=semb_d[:],
                    in_offset=bass.IndirectOffsetOnAxis(
                        ap=tok_sb_idx[:, tc4:tc4 + 1], axis=0))
                sarg = wrk.tile([P, 16], f32, tag="sarg")
                nc.vector.tensor_tensor(sarg[:], sa_ps[:, 0:16], emb[:], ALU.add)
                sens = wrk.tile([P, 16], f32, tag="sens")
                nc.scalar.activation(sens[:], sarg[:], AF.Sigmoid)
                d = wrk.tile([P, 64], f32, tag="d")
                nc.vector.tensor_tensor(d[:], y_tok[tc4][:], x_tok[tc4][:], ALU.subtract)
                dm = wrk.tile([P, 64], f32, tag="dm")
                nc.vector.tensor_tensor(
                    dm[:].rearrange("p (g c) -> p g c", c=4),
                    d[:].rearrange("p (g c) -> p g c", c=4),
                    sens[:, :, None].broadcast_to([P, 16, 4]), ALU.mult)
                o = wrk.tile([P, 64], f32, tag="o")
                nc.vector.tensor_tensor(o[:], x_tok[tc4][:], dm[:], ALU.add)
                nc.gpsimd.dma_start(out_d[tc4 * P:(tc4 + 1) * P, :], o[:])

    _legalize_waits(nc)
    return nc


def _to_blocks(M):
    Bb, Tt = M.shape[:2]
    return np.ascontiguousarray(
        np.transpose(M.reshape(Bb, Tt, 4, 2, 4, 2), (0, 1, 2, 4, 3, 5))
    ).reshape(Bb, Tt, 64)


def _from_blocks(x):
    Bb, Tt = x.shape[:2]
    return np.ascontiguousarray(
        np.transpose(x.reshape(Bb, Tt, 4, 4, 2, 2), (0, 1, 2, 4, 3, 5))
    ).reshape(Bb, Tt, 8, 8)


def _host_params(inp):
    Wqkv_blk = np.asarray(inp['Wqkv_blk'], np.float32)
    bqkv_blk = np.asarray(inp['bqkv_blk'], np.float32)
    Wqkv_c = np.asarray(inp['Wqkv_c'], np.float32)
    bqkv_c = np.asarray(inp['bqkv_c'], np.float32)
    Wo_blk = np.asarray(inp['Wo_blk'], np.float32)
    bo_blk = np.asarray(inp['bo_blk'], np.float32)
    Wo_c = np.asarray(inp['Wo_c'], np.float32)
    bo_c = np.asarray(inp['bo_c'], np.float32)
    W1 = np.asarray(inp['W1'], np.float32)
    b1 = np.asarray(inp['b1'], np.float32)
    W2 = np.asarray(inp['W2'], np.float32)
    b2 = np.asarray(inp['b2'], np.float32)

    # Q/K projection lhsT tiles: pair p=4j+s at columns [32s, 32s+2)
    wq = np.zeros((8, 65, 128), np.float32)
    wk = np.zeros((8, 65, 128), np.float32)
    for pp in range(32):
        j, s = pp // 4, pp % 4
        blk, hh = pp // 2, pp % 2
        for d in range(2):
            wq[j, 4 * blk:4 * blk + 4, 32 * s + d] = Wqkv_blk[blk, 2 * hh + d, :]
            wq[j, 64, 32 * s + d] = bqkv_blk[blk, 2 * hh + d]
            wk[j, 4 * blk:4 * blk + 4, 32 * s + d] = Wqkv_blk[blk, 4 + 2 * hh + d, :]
            wk[j, 64, 32 * s + d] = bqkv_blk[blk, 4 + 2 * hh + d]

    wv96 = np.zeros((65, 96), np.float32)
    for pp in range(32):
        blk, hh = pp // 2, pp % 2
        for d in range(2):
            wv96[4 * blk:4 * blk + 4, 3 * pp + d] = Wqkv_blk[blk, 8 + 2 * hh + d, :]
            wv96[64, 3 * pp + d] = bqkv_blk[blk, 8 + 2 * hh + d]
        wv96[64, 3 * pp + 2] = 1.0

    # cross Q/K lhsT tiles: head h=4j+s at columns [32s, 32s+8)
    wcq = np.zeros((2, 65, 128), np.float32)
    wck = np.zeros((2, 65, 128), np.float32)
    for h in range(8):
        j, s = h // 4, h % 4
        wcq[j, :64, 32 * s:32 * s + 8] = Wqkv_c[8 * h:8 * h + 8, :].T
        wcq[j, 64, 32 * s:32 * s + 8] = bqkv_c[8 * h:8 * h + 8]
        wck[j, :64, 32 * s:32 * s + 8] = Wqkv_c[64 + 8 * h:64 + 8 * h + 8, :].T
        wck[j, 64, 32 * s:32 * s + 8] = bqkv_c[64 + 8 * h:64 + 8 * h + 8]

    wcv72 = np.zeros((65, 72), np.float32)
    for h in range(8):
        wcv72[:64, 9 * h:9 * h + 8] = Wqkv_c[128 + 8 * h:128 + 8 * h + 8, :].T
        wcv72[64, 9 * h:9 * h + 8] = bqkv_c[128 + 8 * h:128 + 8 * h + 8]
        wcv72[64, 9 * h + 8] = 1.0

    wobig = np.zeros((65, 64), np.float32)
    for blk in range(NBLK):
        wobig[4 * blk:4 * blk + 4, 4 * blk:4 * blk + 4] = Wo_blk[blk].T
        wobig[64, 4 * blk:4 * blk + 4] = bo_blk[blk]

    woc = np.zeros((65, 64), np.float32)
    woc[:64, :] = Wo_c.T
    woc[64, :] = bo_c
    w1 = np.zeros((65, 256), np.float32)
    w1[:64, :] = W1.T
    w1[64, :] = b1
    w2 = np.ascontiguousarray(W2.T).reshape(2, 128, 64)
    lnp = np.stack([np.asarray(inp['g1'], np.float32), np.asarray(inp['be1'], np.float32),
                    np.asarray(inp['g2'], np.float32), np.asarray(inp['be2'], np.float32)])
    return dict(wq=wq, wk=wk, wv96=wv96, wcq=wcq, wck=wck, wcv72=wcv72,
                wobig=wobig, woc=woc, w1=w1, w2=w2,
                b2=b2.reshape(64, 1).copy(),
                lnp=np.ascontiguousarray(lnp),
                alpha=np.asarray(inp['sens_alpha'], np.float32).reshape(1, 16).copy(),
                semb=np.asarray(inp['sens_emb'], np.float32),
                ident=np.eye(128, dtype=np.float32),
                e4=np.stack([np.eye(4, dtype=np.float32)[t].reshape(4, 1)
                             * np.ones((1, 128), np.float32) for t in range(4)]))


_NC_CACHE = None


def kernel(**inputs):
    global _NC_CACHE
    M = np.asarray(inputs['M'], np.float32)
    tok = np.asarray(inputs['token_ids']).astype(np.int64)
    params = _host_params(inputs)

    blocks = _to_blocks(M)                       # [B, T, 64]
    in_maps = []
    for c in range(8):
        b, h = c // 2, c % 2
        xt = np.concatenate([blocks[b].T, np.ones((1, T), np.float32)], axis=0)
        m = dict(params)
        m['xt_aug'] = np.ascontiguousarray(xt)
        m['xtq_aug'] = np.ascontiguousarray(xt[:, h * TQ:(h + 1) * TQ])
        m['x_tok'] = np.ascontiguousarray(blocks[b, h * TQ:(h + 1) * TQ, :])
        ids = tok[b, h * TQ:(h + 1) * TQ].astype(np.int32)
        m['tokidx'] = np.ascontiguousarray(ids.reshape(4, P).T)  # [p, c] : t=c*128+p
        in_maps.append(m)

    if _NC_CACHE is None:
        _NC_CACHE = build_program()
    res = run_bass_kernel_spmd(_NC_CACHE, in_maps, core_ids=list(range(8))).results

    out_blocks = np.zeros((B, T, 64), np.float32)
    for c in range(8):
        b, h = c // 2, c % 2
        out_blocks[b, h * TQ:(h + 1) * TQ, :] = res[c]['out']
    return _from_blocks(out_blocks.reshape(B, T, 16, 4))


if __name__ == '__main__':
    import reference as ref
    import jax
    inp = {k: np.asarray(jax.device_get(v)) for k, v in ref.setup_inputs().items()}
    got = kernel(**inp)
    exp = np.asarray(jax.device_get(ref.reference(**{k: v for k, v in inp.items()})))
    err = np.abs(got - exp)
    print(f"max abs {err.max():.3e}  scale-rel {err.max()/np.abs(exp).max():.3e}")



# revision 7
# speedup vs baseline: 15.9357x; 15.9357x over previous
"""Trainium2 Bass kernel for nn_BlockWiseSemanticAttention_944892805640.

Sharding: 8 cores = (batch b = c//2) x (T-half h = c%2). Each core computes
its 512 query rows of one batch element end-to-end; keys/values span the
full T=1024. Two tiny pairwise collectives: AllGather of the block-attention
output (cross-attn K/V needs full T) and ReduceScatter of the attention-
received partial sums.

Device algorithm (validated in numpy proto against the jax reference):
  blocks = to_blocks(M)  (host, pure permutation)
  per (blk,head) pair p of 32: S.T = k_p^T q_p (PE, f32r), U = exp(S/sqrt2)
  (ACT), [o0,o1,r] = [v_p|1]^T U (PE accum).  Normalize token-major, apply
  block-diag Wo_blk, AllGather -> cross attention (8 heads, same scheme,
  plus attn-received partials via DVE mult+reduce), Wo_c, LN1, FFN (gelu),
  LN2, sens = sigmoid(emb[token] + recv*alpha), out = x + (y-x)*sens.

f32r (TF32-like, 1 cyc/row) is used for all T^2 matmuls; fp32 elsewhere.
Workarounds for this toolchain (measured on HW):
  - matmul operand base partitions must be 0/32/64/96 -> Q/K tiles place
    each pair at a 32-aligned partition slot (free: it is just lhsT column
    ordering in the QKV projection matmuls).
  - walrus accepts at most ONE sync-wait per instruction -> post-pass splits
    extra waits onto same-engine NoOps.
  - SWDGE queue >0 semaphores never fire under this runtime ->
    num_swdge_queues=1 and all DMA on gpsimd.
  - f32r operands must be produced by rounding compute ops (DVE/ACT writes).
"""
import sys

sys.path.insert(0, '/opt/trn_rl_repo')

import numpy as np

import concourse.bass as bass
import concourse.tile as tile
from concourse import mybir
from concourse.bass_utils import run_bass_kernel_spmd

AF = mybir.ActivationFunctionType
ALU = mybir.AluOpType
AX = mybir.AxisListType
f32 = mybir.dt.float32
f32r = mybir.dt.float32r
i32 = mybir.dt.int32

B, T, TQ, P = 4, 1024, 512, 128
NBLK = 16
LN_EPS = 1e-5
INV_SQRT2 = 0.7071067811865476
INV_SQRT8 = 0.35355339059327373
REPLICA_GROUPS = [[0, 1], [2, 3], [4, 5], [6, 7]]


def _legalize_waits(nc):
    """This walrus build accepts at most ONE sync-wait per instruction.
    Split extras onto same-engine NoOps placed immediately before (engine
    streams execute in block order, so this is safe)."""
    n_split = 0
    for func in nc.m.functions:
        for bb in func.blocks:
            out, changed = [], False
            for inst in bb.instructions:
                si = inst.sync_info
                if si is not None and si.on_wait is not None and len(si.on_wait) > 1:
                    waits = list(si.on_wait)
                    for i, w in enumerate(waits[:-1]):
                        nop = mybir.InstNoOp(name=f"{inst.name}-lw{i}", ins=[], outs=[])
                        nop.engine = inst.engine
                        nop.sync_info = mybir.SyncInfo(on_wait=[w], on_update=[])
                        out.append(nop)
                    inst.sync_info = mybir.SyncInfo(
                        on_wait=[waits[-1]], on_update=list(si.on_update or []))
                    changed = True
                    n_split += 1
                upd = inst.sync_info.on_update if inst.sync_info else None
                assert not upd or len(upd) <= 1, f"multi-update on {inst.name}"
                out.append(inst)
            if changed:
                bb.instructions = out
    return n_split


def build_program():
    nc = bass.Bass(num_swdge_queues=1)

    # ---- I/O ----
    xt_aug_d = nc.dram_tensor("xt_aug", [65, T], f32, kind="ExternalInput")
    xtq_aug_d = nc.dram_tensor("xtq_aug", [65, TQ], f32, kind="ExternalInput")
    x_tok_d = nc.dram_tensor("x_tok", [TQ, 64], f32, kind="ExternalInput")
    wq_d = nc.dram_tensor("wq", [8, 65, 128], f32, kind="ExternalInput")
    wk_d = nc.dram_tensor("wk", [8, 65, 128], f32, kind="ExternalInput")
    wv96_d = nc.dram_tensor("wv96", [65, 96], f32, kind="ExternalInput")
    wcq_d = nc.dram_tensor("wcq", [2, 65, 128], f32, kind="ExternalInput")
    wck_d = nc.dram_tensor("wck", [2, 65, 128], f32, kind="ExternalInput")
    wcv72_d = nc.dram_tensor("wcv72", [65, 72], f32, kind="ExternalInput")
    wobig_d = nc.dram_tensor("wobig", [65, 64], f32, kind="ExternalInput")
    woc_d = nc.dram_tensor("woc", [65, 64], f32, kind="ExternalInput")
    w1_d = nc.dram_tensor("w1", [65, 256], f32, kind="ExternalInput")
    w2_d = nc.dram_tensor("w2", [2, 128, 64], f32, kind="ExternalInput")
    b2_d = nc.dram_tensor("b2", [64, 1], f32, kind="ExternalInput")
    lnp_d = nc.dram_tensor("lnp", [4, 64], f32, kind="ExternalInput")  # g1,be1,g2,be2
    alpha_d = nc.dram_tensor("alpha", [1, 16], f32, kind="ExternalInput")
    semb_d = nc.dram_tensor("semb", [32000, 16], f32, kind="ExternalInput")
    tokidx_d = nc.dram_tensor("tokidx", [P, 4], i32, kind="ExternalInput")
    ident_d = nc.dram_tensor("ident", [P, P], f32, kind="ExternalInput")
    e4_d = nc.dram_tensor("e4", [4, 4, P], f32, kind="ExternalInput")
    out_d = nc.dram_tensor("out", [TQ, 64], f32, kind="ExternalOutput")

    # internal DRAM for collectives
    ab_half_d = nc.dram_tensor("ab_half", [64, TQ], f32)
    ab_gath_d = nc.dram_tensor("ab_gath", [128, TQ], f32)
    recv_full_d = nc.dram_tensor("recv_full", [T], f32)
    recv_rs_d = nc.dram_tensor("recv_rs", [TQ], f32)

    with tile.TileContext(nc) as tc:
        with tc.tile_pool(name="const", bufs=1) as cst, \
             tc.tile_pool(name="work", bufs=1) as wrk, \
             tc.tile_pool(name="ut", bufs=10) as utp, \
             tc.tile_pool(name="sc2", bufs=2) as sc2, \
             tc.tile_pool(name="ps_score", bufs=2, space="PSUM") as pss, \
             tc.tile_pool(name="ps_acc", bufs=2, space="PSUM") as psa, \
             tc.tile_pool(name="ps_tok", bufs=1, space="PSUM") as pstok, \
             tc.tile_pool(name="ps_misc", bufs=1, space="PSUM") as psm:

            # ================= load constants =================
            def dmain(shape, dt, src, tag):
                t = cst.tile(shape, dt, tag=tag)
                nc.gpsimd.dma_start(t[:], src)
                return t

            xt_sb = dmain([65, T], f32, xt_aug_d[:], "xt")
            xtq_sb = dmain([65, TQ], f32, xtq_aug_d[:], "xtq")
            ident = dmain([P, P], f32, ident_d[:], "ident")
            wv96_sb = dmain([65, 96], f32, wv96_d[:], "wv96")
            wcv72_sb = dmain([65, 72], f32, wcv72_d[:], "wcv72")
            wobig_sb = dmain([65, 64], f32, wobig_d[:], "wobig")
            woc_sb = dmain([65, 64], f32, woc_d[:], "woc")
            w1_sb = dmain([65, 256], f32, w1_d[:], "w1")
            w2_sb = [dmain([P, 64], f32, w2_d[m], f"w2_{m}") for m in range(2)]
            b2_sb = dmain([64, 1], f32, b2_d[:], "b2")
            alpha_sb = dmain([1, 16], f32, alpha_d[:], "alpha")
            tok_sb_idx = dmain([P, 4], i32, tokidx_d[:], "tokidx")
            x_tok = [dmain([P, 64], f32, x_tok_d[c * P:(c + 1) * P, :], f"xtok{c}")
                     for c in range(4)]
            wq_sb = [dmain([65, 128], f32, wq_d[j], f"wq{j}") for j in range(8)]
            wk_sb = [dmain([65, 128], f32, wk_d[j], f"wk{j}") for j in range(8)]
            wcq_sb = [dmain([65, 128], f32, wcq_d[j], f"wcq{j}") for j in range(2)]
            e4_sb = [dmain([4, P], f32, e4_d[t], f"e4_{t}") for t in range(4)]
            wck_sb = [dmain([65, 128], f32, wck_d[j], f"wck{j}") for j in range(2)]
            # LN params broadcast to [P, 64]
            lnb = []
            for i in range(4):
                t = cst.tile([P, 64], f32, tag=f"lnb{i}")
                nc.gpsimd.dma_start(t[:], lnp_d[i:i + 1, :].broadcast_to([P, 64]))
                lnb.append(t)
            g1b, be1b, g2b, be2b = lnb
            # bo_blk broadcast (row 64 of wobig)
            bob = cst.tile([P, 64], f32, tag="bob")
            nc.gpsimd.dma_start(bob[:], wobig_d[64:65, :].broadcast_to([P, 64]))

            eps_p1 = cst.tile([P, 1], f32, tag="eps")
            nc.vector.memset(eps_p1[:], LN_EPS)
            ones1 = cst.tile([1, P], f32, tag="ones1")
            nc.vector.memset(ones1[:], 1.0)

            # f32r roundings of weights/inputs feeding f32r matmuls
            def to_r(src, shape, tag):
                t = cst.tile(shape, f32r, tag=tag)
                nc.vector.tensor_copy(t[:], src[:])
                return t

            xt_r = to_r(xt_sb, [65, T], "xt_r")
            xtq_r = to_r(xtq_sb, [65, TQ], "xtq_r")
            wq_r = [to_r(wq_sb[j], [65, 128], f"wq_r{j}") for j in range(8)]
            wk_r = [to_r(wk_sb[j], [65, 128], f"wk_r{j}") for j in range(8)]
            wv96_r = to_r(wv96_sb, [65, 96], "wv96_r")
            wcq_r = [to_r(wcq_sb[j], [65, 128], f"wcq_r{j}") for j in range(2)]
            wck_r = [to_r(wck_sb[j], [65, 128], f"wck_r{j}") for j in range(2)]
            wcv72_r = to_r(wcv72_sb, [65, 72], "wcv72_r")

            # ================= phase 1: block QKV =================
            # Q tiles: pair p=4j+s at partitions [32s, 32s+2)
            QT = [wrk.tile([P, TQ], f32r, tag=f"QT{j}", name=f"QT{j}")
                  for j in range(8)]
            for jq in range(4):
                q_ps = pss.tile([P, 2 * TQ], f32, tag="s", name=f"qps{jq}")
                for c in range(2):
                    nc.tensor.matmul(q_ps[:, c * TQ:(c + 1) * TQ],
                                     wq_r[2 * jq + c][:], xtq_r[:],
                                     start=True, stop=True)
                    nc.vector.tensor_copy(QT[2 * jq + c][:],
                                          q_ps[:, c * TQ:(c + 1) * TQ])
            KT = []
            for j in range(8):
                t = wrk.tile([P, T], f32r, tag=f"KT{j}")
                k_ps = pss.tile([P, 2 * TQ], f32, tag="s", name=f"kps{j}")
                for c in range(2):
                    nc.tensor.matmul(k_ps[:, c * TQ:(c + 1) * TQ], wk_r[j][:],
                                     xt_r[:, c * TQ:(c + 1) * TQ],
                                     start=True, stop=True)
                nc.vector.tensor_copy(t[:], k_ps[:])
                KT.append(t)
            VA = [wrk.tile([P, 96], f32r, tag=f"VA{kc}", name=f"VA{kc}")
                  for kc in range(8)]
            for kv in range(4):
                v_ps = pss.tile([P, 2 * TQ], f32, tag="s", name=f"vps{kv}")
                for c in range(2):
                    kc = 2 * kv + c
                    nc.tensor.matmul(v_ps[:, c * TQ:c * TQ + 96],
                                     xt_r[:, kc * P:(kc + 1) * P], wv96_r[:],
                                     start=True, stop=True)
                    nc.vector.tensor_copy(VA[kc][:], v_ps[:, c * TQ:c * TQ + 96])

            # ================= phase 2: block attention =================
            # token-major accumulator of transposed AV results:
            # col 96*t4 + 3*p + {0,1,2} = (o0, o1, r) of pair p, token block t4
            tok_ps = pstok.tile([P, 384], f32, tag="tok", name="tok_ps")
            for j in range(8):
                for sp_i in range(2):          # two pairs per score tile
                    sa, sb_ = 2 * sp_i, 2 * sp_i + 1
                    pa, pb = 4 * j + sa, 4 * j + sb_
                    av_a = psa.tile([3, TQ], f32, tag="av")
                    av_b = psa.tile([3, TQ], f32, tag="av")
                    for kc in range(8):
                        s_ps = pss.tile([P, 2 * TQ], f32, tag="s")
                        nc.tensor.matmul(s_ps[:, 0:TQ],
                                         KT[j][32 * sa:32 * sa + 2, kc * P:(kc + 1) * P],
                                         QT[j][32 * sa:32 * sa + 2, :],
                                         start=True, stop=True,
                                         tile_position=(32 * sa, 0))
                        nc.tensor.matmul(s_ps[:, TQ:2 * TQ],
                                         KT[j][32 * sb_:32 * sb_ + 2, kc * P:(kc + 1) * P],
                                         QT[j][32 * sb_:32 * sb_ + 2, :],
                                         start=True, stop=True,
                                         tile_position=(32 * sb_, 0))
                        ut = utp.tile([P, 2 * TQ], f32r, tag="ut")
                        nc.scalar.activation(ut[:], s_ps[:], AF.Exp, scale=INV_SQRT2)
                        nc.tensor.matmul(av_a[:], VA[kc][:, 3 * pa:3 * pa + 3],
                                         ut[:, 0:TQ],
                                         start=(kc == 0), stop=(kc == 7))
                        nc.tensor.matmul(av_b[:], VA[kc][:, 3 * pb:3 * pb + 3],
                                         ut[:, TQ:2 * TQ],
                                         start=(kc == 0), stop=(kc == 7))
                    for pp, av in ((pa, av_a), (pb, av_b)):
                        avs = sc2.tile([3, TQ], f32, tag="avsb", name=f"avs{pp}")
                        nc.vector.tensor_copy(avs[:], av[:])
                        for t4 in range(4):
                            nc.tensor.matmul(
                                tok_ps[:, 96 * t4 + 3 * pp:96 * t4 + 3 * pp + 3],
                                avs[:, t4 * P:(t4 + 1) * P], ident[0:3, 0:3],
                                is_transpose=True, start=True, stop=True)

            # ================= phase 3: normalize + project + gather =================
            abT_aug = wrk.tile([65, TQ], f32, tag="abT")
            nc.vector.memset(abT_aug[64:65, :], 1.0)
            for tc4 in range(4):
                tok96 = wrk.tile([P, 96], f32, tag="tok96")
                nc.vector.tensor_copy(tok96[:], tok_ps[:, 96 * tc4:96 * tc4 + 96])
                nc.vector.reciprocal(tok96[:, 2:96:3], tok96[:, 2:96:3])
                abpre = wrk.tile([P, 64], f32, tag="abpre")
                g3 = tok96[:].rearrange("p (g c) -> p g c", c=3)
                nc.vector.tensor_tensor(
                    abpre[:].rearrange("p (g c) -> p g c", c=2),
                    g3[:, :, 0:2], g3[:, :, 2:3].broadcast_to([P, 32, 2]), ALU.mult)
                tb_ps = psm.tile([P, TQ], f32, tag="m")
                nc.tensor.matmul(tb_ps[0:64, 0:P], abpre[:], ident[:, :],
                                 is_transpose=True, start=True, stop=True)
                nc.vector.tensor_copy(abT_aug[0:64, tc4 * P:(tc4 + 1) * P],
                                      tb_ps[0:64, 0:P])

            # feature-major projected half (for AllGather / cross K,V)
            pj_ps = psm.tile([P, TQ], f32, tag="m")
            nc.tensor.matmul(pj_ps[0:64, :], wobig_sb[:], abT_aug[:],
                             start=True, stop=True)
            abp_sb = wrk.tile([64, TQ], f32, tag="abp")
            nc.vector.tensor_copy(abp_sb[:], pj_ps[0:64, :])
            nc.gpsimd.dma_start(ab_half_d[:], abp_sb[:])
            nc.gpsimd.collective_compute(
                "AllGather", ALU.bypass, replica_groups=REPLICA_GROUPS,
                ins=[ab_half_d[:].opt()], outs=[ab_gath_d[:].opt()])

            # token-major projected (+ bo) for the residual path
            ab_tok = []
            for tc4 in range(4):
                pt_ps = psm.tile([P, TQ], f32, tag="m")
                nc.tensor.matmul(pt_ps[:, 0:64], abT_aug[0:64, tc4 * P:(tc4 + 1) * P],
                                 wobig_sb[0:64, :], start=True, stop=True)
                t = wrk.tile([P, 64], f32, tag=f"abtok{tc4}")
                nc.vector.tensor_tensor(t[:], pt_ps[:, 0:64], bob[:], ALU.add)
                ab_tok.append(t)

            # assemble full all_blocksT (+ones) and q-side (+ones), f32r copies
            abf_sb = wrk.tile([65, T], f32, tag="abf")
            nc.vector.memset(abf_sb[64:65, :], 1.0)
            nc.gpsimd.dma_start(abf_sb[0:64, 0:TQ], ab_gath_d[0:64, :])
            nc.gpsimd.dma_start(abf_sb[0:64, TQ:T], ab_gath_d[64:128, :])
            abq_sb = wrk.tile([65, TQ], f32, tag="abq")
            nc.vector.memset(abq_sb[64:65, :], 1.0)
            nc.vector.tensor_copy(abq_sb[0:64, :], abp_sb[:])
            abf_r = wrk.tile([65, T], f32r, tag="abf_r")
            nc.vector.tensor_copy(abf_r[:], abf_sb[:])
            abq_r = wrk.tile([65, TQ], f32r, tag="abq_r")
            nc.vector.tensor_copy(abq_r[:], abq_sb[:])

            # ================= phase 4: cross attention =================
            QC = [wrk.tile([P, TQ], f32r, tag=f"QC{j}", name=f"QC{j}")
                  for j in range(2)]
            qc_ps = pss.tile([P, 2 * TQ], f32, tag="s", name="qcps")
            for j in range(2):
                nc.tensor.matmul(qc_ps[:, j * TQ:(j + 1) * TQ], wcq_r[j][:],
                                 abq_r[:], start=True, stop=True)
                nc.vector.tensor_copy(QC[j][:], qc_ps[:, j * TQ:(j + 1) * TQ])
            KC = []
            for j in range(2):
                t = wrk.tile([P, T], f32r, tag=f"KC{j}")
                k_ps = pss.tile([P, 2 * TQ], f32, tag="s", name=f"kcps{j}")
                for c in range(2):
                    nc.tensor.matmul(k_ps[:, c * TQ:(c + 1) * TQ], wck_r[j][:],
                                     abf_r[:, c * TQ:(c + 1) * TQ],
                                     start=True, stop=True)
                nc.vector.tensor_copy(t[:], k_ps[:])
                KC.append(t)
            VC = [wrk.tile([P, 72], f32r, tag=f"VC{kc}", name=f"VC{kc}")
                  for kc in range(8)]
            for kv in range(4):
                v_ps = pss.tile([P, 2 * TQ], f32, tag="s", name=f"vcps{kv}")
                for c in range(2):
                    kc = 2 * kv + c
                    nc.tensor.matmul(v_ps[:, c * TQ:c * TQ + 72],
                                     abf_r[:, kc * P:(kc + 1) * P], wcv72_r[:],
                                     start=True, stop=True)
                    nc.vector.tensor_copy(VC[kc][:], v_ps[:, c * TQ:c * TQ + 72])

            # token-major accumulator: col 96*t4 + 9*h + {0..7, 8} = (vals, r)
            ctok_ps = pstok.tile([P, 384], f32, tag="tok", name="ctok_ps")
            racc = [wrk.tile([P, 8], f32, tag=f"racc{kc}", name=f"racc{kc}") for kc in range(8)]

            for jj in range(2):
                for sp_i in range(2):
                    sa, sb_ = 2 * sp_i, 2 * sp_i + 1
                    ha, hb = 4 * jj + sa, 4 * jj + sb_
                    avh = {ha: psa.tile([9, TQ], f32, tag="av", name=f"avh{ha}"),
                           hb: psa.tile([9, TQ], f32, tag="av", name=f"avh{hb}")}
                    utc = []
                    for kc in range(8):
                        s_ps = pss.tile([P, 2 * TQ], f32, tag="s")
                        nc.tensor.matmul(s_ps[:, 0:TQ],
                                         KC[jj][32 * sa:32 * sa + 8, kc * P:(kc + 1) * P],
                                         QC[jj][32 * sa:32 * sa + 8, :],
                                         start=True, stop=True,
                                         tile_position=(32 * sa, 0))
                        nc.tensor.matmul(s_ps[:, TQ:2 * TQ],
                                         KC[jj][32 * sb_:32 * sb_ + 8, kc * P:(kc + 1) * P],
                                         QC[jj][32 * sb_:32 * sb_ + 8, :],
                                         start=True, stop=True,
                                         tile_position=(32 * sb_, 0))
                        ut = utp.tile([P, 2 * TQ], f32r, tag="ut")
                        nc.scalar.activation(ut[:], s_ps[:], AF.Exp, scale=INV_SQRT8)
                        utc.append(ut)
                        for hh, off in ((ha, 0), (hb, TQ)):
                            nc.tensor.matmul(avh[hh][:],
                                             VC[kc][:, 9 * hh:9 * hh + 9],
                                             ut[:, off:off + TQ],
                                             start=(kc == 0), stop=(kc == 7))
                    for hh, off in ((ha, 0), (hb, TQ)):
                        avs = sc2.tile([9, TQ], f32, tag="avsc", name=f"avsc{hh}")
                        nc.vector.tensor_copy(avs[:], avh[hh][:])
                        rtok4 = wrk.tile([P, 4], f32, tag="rtok")
                        for t4 in range(4):
                            nc.tensor.matmul(
                                ctok_ps[:, 96 * t4 + 9 * hh:96 * t4 + 9 * hh + 9],
                                avs[:, t4 * P:(t4 + 1) * P], ident[0:9, 0:9],
                                is_transpose=True, start=True, stop=True)
                            nc.vector.reciprocal(
                                rtok4[:, t4:t4 + 1],
                                ctok_ps[:, 96 * t4 + 9 * hh + 8:96 * t4 + 9 * hh + 9])
                        # rebuild feature-major invr and broadcast across partitions
                        rt_ps = psm.tile([P, TQ], f32, tag="m")
                        nc.tensor.matmul(rt_ps[0:4, 0:P], rtok4[:], ident[:, :],
                                         is_transpose=True, start=True, stop=True)
                        rfeat = sc2.tile([4, P], f32, tag="rfeat")
                        nc.vector.tensor_copy(rfeat[:], rt_ps[0:4, 0:P])
                        ib_ps = psm.tile([P, TQ], f32, tag="m")
                        for t4 in range(4):
                            nc.tensor.matmul(ib_ps[:, t4 * P:(t4 + 1) * P],
                                             e4_sb[t4][:], rfeat[:],
                                             start=True, stop=True)
                        invrb = sc2.tile([P, TQ], f32, tag="invrb")
                        nc.vector.tensor_copy(invrb[:], ib_ps[:])
                        for kc in range(8):
                            scr = sc2.tile([P, TQ], f32, tag="scr")
                            nc.vector.tensor_tensor(
                                scr[:], utc[kc][:, off:off + TQ].bitcast(f32),
                                invrb[:], ALU.mult)
                            nc.vector.reduce_sum(racc[kc][:, hh:hh + 1], scr[:],
                                                 axis=AX.X)

            # normalize token-major, then transpose back to feature-major
            crossT_aug = wrk.tile([65, TQ], f32, tag="crossT")
            nc.vector.memset(crossT_aug[64:65, :], 1.0)
            for t4 in range(4):
                ctok96 = wrk.tile([P, 96], f32, tag="ctok96")
                nc.vector.tensor_copy(ctok96[:, 0:72], ctok_ps[:, 96 * t4:96 * t4 + 72])
                nc.vector.reciprocal(ctok96[:, 8:72:9], ctok96[:, 8:72:9])
                atok = wrk.tile([P, 64], f32, tag="atok")
                g9 = ctok96[:, 0:72].rearrange("p (g c) -> p g c", c=9)
                nc.vector.tensor_tensor(
                    atok[:].rearrange("p (g c) -> p g c", c=8),
                    g9[:, :, 0:8], g9[:, :, 8:9].broadcast_to([P, 8, 8]), ALU.mult)
                cb_ps = psm.tile([P, TQ], f32, tag="m")
                nc.tensor.matmul(cb_ps[0:64, 0:P], atok[:], ident[:, :],
                                 is_transpose=True, start=True, stop=True)
                nc.vector.tensor_copy(crossT_aug[0:64, t4 * P:(t4 + 1) * P],
                                      cb_ps[0:64, 0:P])

            # recv partials -> DRAM -> ReduceScatter -> own 512 slice
            rsum = wrk.tile([P, 8], f32, tag="rsum")
            for kc in range(8):
                nc.vector.reduce_sum(rsum[:, kc:kc + 1], racc[kc][:], axis=AX.X)
                nc.gpsimd.dma_start(recv_full_d[kc * P:(kc + 1) * P],
                                    rsum[:, kc:kc + 1])
            nc.gpsimd.collective_compute(
                "ReduceScatter", ALU.add, replica_groups=REPLICA_GROUPS,
                ins=[recv_full_d[:].opt()], outs=[recv_rs_d[:].opt()])
            recv_sb = wrk.tile([1, TQ], f32, tag="recv")
            nc.gpsimd.dma_start(recv_sb[:], recv_rs_d[None, :])
            nc.vector.tensor_scalar_mul(recv_sb[:], recv_sb[:], 1.0 / 8192.0)

            # cross output projection (feature-major)
            co_ps = psm.tile([P, TQ], f32, tag="m")
            nc.tensor.matmul(co_ps[0:64, :], woc_sb[:], crossT_aug[:],
                             start=True, stop=True)
            co_sb = wrk.tile([64, TQ], f32, tag="co")
            nc.vector.tensor_copy(co_sb[:], co_ps[0:64, :])

            # ============== phase 5/6: resid+LN1, FFN, resid+LN2 ==============
            def layernorm(x_t, gb, beb, tag, ytag):
                s1 = wrk.tile([P, 1], f32, tag=f"{tag}s1")
                nc.vector.reduce_sum(s1[:], x_t[:], axis=AX.X)
                nc.scalar.mul(s1[:], s1[:], -1.0 / 64.0)
                xc = wrk.tile([P, 64], f32, tag=f"{tag}xc")
                nc.scalar.add(xc[:], x_t[:], s1[:])
                sq = wrk.tile([P, 64], f32, tag=f"{tag}sq")
                nc.scalar.activation(sq[:], xc[:], AF.Square)
                var = wrk.tile([P, 1], f32, tag=f"{tag}v")
                nc.vector.reduce_sum(var[:], sq[:], axis=AX.X)
                nc.scalar.mul(var[:], var[:], 1.0 / 64.0)
                sd = wrk.tile([P, 1], f32, tag=f"{tag}sd")
                nc.scalar.activation(sd[:], var[:], AF.Sqrt, bias=eps_p1[:])
                rstd = wrk.tile([P, 1], f32, tag=f"{tag}rs")
                nc.vector.reciprocal(rstd[:], sd[:])
                y = wrk.tile([P, 64], f32, tag=ytag)
                nc.vector.tensor_scalar_mul(y[:], xc[:], rstd[:])
                nc.vector.tensor_tensor(y[:], y[:], gb[:], ALU.mult)
                nc.vector.tensor_tensor(y[:], y[:], beb[:], ALU.add)
                return y

            ln1_tok = []
            ln1T_aug = wrk.tile([65, TQ], f32, tag="ln1T")
            nc.vector.memset(ln1T_aug[64:65, :], 1.0)
            for tc4 in range(4):
                ct_ps = psm.tile([P, TQ], f32, tag="m")
                nc.tensor.matmul(ct_ps[:, 0:64], co_sb[:, tc4 * P:(tc4 + 1) * P],
                                 ident[0:64, 0:64], is_transpose=True,
                                 start=True, stop=True)
                x1 = wrk.tile([P, 64], f32, tag="x1")
                nc.vector.tensor_tensor(x1[:], ct_ps[:, 0:64], ab_tok[tc4][:], ALU.add)
                y = layernorm(x1, g1b, be1b, "ln1", f"ln1y{tc4}")
                ln1_tok.append(y)
                lt_ps = psm.tile([P, TQ], f32, tag="m")
                nc.tensor.matmul(lt_ps[0:64, 0:P], y[:], ident[:, :],
                                 is_transpose=True, start=True, stop=True)
                nc.vector.tensor_copy(ln1T_aug[0:64, tc4 * P:(tc4 + 1) * P],
                                      lt_ps[0:64, 0:P])

            # FFN: h = gelu(W1 @ ln1T); o2 = W2 @ h + b2
            h_sb = []
            for m in range(2):
                h_ps = psm.tile([P, TQ], f32, tag="m")
                nc.tensor.matmul(h_ps[:], w1_sb[:, m * P:(m + 1) * P], ln1T_aug[:],
                                 start=True, stop=True)
                t = wrk.tile([P, TQ], f32, tag=f"h{m}")
                nc.scalar.activation(t[:], h_ps[:], AF.Gelu)
                h_sb.append(t)
            o2_ps = psm.tile([P, TQ], f32, tag="m")
            for m in range(2):
                nc.tensor.matmul(o2_ps[0:64, :], w2_sb[m][:], h_sb[m][:],
                                 start=(m == 0), stop=(m == 1))
            o2_sb = wrk.tile([64, TQ], f32, tag="o2")
            nc.scalar.activation(o2_sb[:], o2_ps[0:64, :], AF.Identity, bias=b2_sb[:])

            y_tok = []
            for tc4 in range(4):
                ot_ps = psm.tile([P, TQ], f32, tag="m")
                nc.tensor.matmul(ot_ps[:, 0:64], o2_sb[:, tc4 * P:(tc4 + 1) * P],
                                 ident[0:64, 0:64], is_transpose=True,
                                 start=True, stop=True)
                x2 = wrk.tile([P, 64], f32, tag="x2")
                nc.vector.tensor_tensor(x2[:], ot_ps[:, 0:64], ln1_tok[tc4][:], ALU.add)
                y_tok.append(layernorm(x2, g2b, be2b, "ln2", f"ln2y{tc4}"))

            # ================= phase 7: sens + combine =================
            for tc4 in range(4):
                sa_ps = psm.tile([P, TQ], f32, tag="m")
                nc.tensor.matmul(sa_ps[:, 0:16], recv_sb[0:1, tc4 * P:(tc4 + 1) * P],
                                 alpha_sb[:], start=True, stop=True)
                emb = wrk.tile([P, 16], f32, tag="emb")
                nc.gpsimd.indirect_dma_start(
                    out=emb[:], out_offset=None, in_=semb_d[:],
                    in_offset=bass.IndirectOffsetOnAxis(
                        ap=tok_sb_idx[:, tc4:tc4 + 1], axis=0))
                sarg = wrk.tile([P, 16], f32, tag="sarg")
                nc.vector.tensor_tensor(sarg[:], sa_ps[:, 0:16], emb[:], ALU.add)
                sens = wrk.tile([P, 16], f32, tag="sens")
                nc.scalar.activation(sens[:], sarg[:], AF.Sigmoid)
                d = wrk.tile([P, 64], f32, tag="d")
                nc.vector.tensor_tensor(d[:], y_tok[tc4][:], x_tok[tc4][:], ALU.subtract)
                dm = wrk.tile([P, 64], f32, tag="dm")
                nc.vector.tensor_tensor(
                    dm[:].rearrange("p (g c) -> p g c", c=4),
                    d[:].rearrange("p (g c) -> p g c", c=4),
                    sens[:, :, None].broadcast_to([P, 16, 4]), ALU.mult)
                o = wrk.tile([P, 64], f32, tag="o")
                nc.vector.tensor_tensor(o[:], x_tok[tc4][:], dm[:], ALU.add)
                nc.gpsimd.dma_start(out_d[tc4 * P:(tc4 + 1) * P, :], o[:])

    _legalize_waits(nc)
    return nc


def _to_blocks(M):
    Bb, Tt = M.shape[:2]
    return np.ascontiguousarray(
        np.transpose(M.reshape(Bb, Tt, 4, 2, 4, 2), (0, 1, 2, 4, 3, 5))
    ).reshape(Bb, Tt, 64)


def _from_blocks(x):
    Bb, Tt = x.shape[:2]
    return np.ascontiguousarray(
        np.transpose(x.reshape(Bb, Tt, 4, 4, 2, 2), (0, 1, 2, 4, 3, 5))
    ).reshape(Bb, Tt, 8, 8)


def _host_params(inp):
    Wqkv_blk = np.asarray(inp['Wqkv_blk'], np.float32)
    bqkv_blk = np.asarray(inp['bqkv_blk'], np.float32)
    Wqkv_c = np.asarray(inp['Wqkv_c'], np.float32)
    bqkv_c = np.asarray(inp['bqkv_c'], np.float32)
    Wo_blk = np.asarray(inp['Wo_blk'], np.float32)
    bo_blk = np.asarray(inp['bo_blk'], np.float32)
    Wo_c = np.asarray(inp['Wo_c'], np.float32)
    bo_c = np.asarray(inp['bo_c'], np.float32)
    W1 = np.asarray(inp['W1'], np.float32)
    b1 = np.asarray(inp['b1'], np.float32)
    W2 = np.asarray(inp['W2'], np.float32)
    b2 = np.asarray(inp['b2'], np.float32)

    # Q/K projection lhsT tiles: pair p=4j+s at columns [32s, 32s+2)
    wq = np.zeros((8, 65, 128), np.float32)
    wk = np.zeros((8, 65, 128), np.float32)
    for pp in range(32):
        j, s = pp // 4, pp % 4
        blk, hh = pp // 2, pp % 2
        for d in range(2):
            wq[j, 4 * blk:4 * blk + 4, 32 * s + d] = Wqkv_blk[blk, 2 * hh + d, :]
            wq[j, 64, 32 * s + d] = bqkv_blk[blk, 2 * hh + d]
            wk[j, 4 * blk:4 * blk + 4, 32 * s + d] = Wqkv_blk[blk, 4 + 2 * hh + d, :]
            wk[j, 64, 32 * s + d] = bqkv_blk[blk, 4 + 2 * hh + d]

    wv96 = np.zeros((65, 96), np.float32)
    for pp in range(32):
        blk, hh = pp // 2, pp % 2
        for d in range(2):
            wv96[4 * blk:4 * blk + 4, 3 * pp + d] = Wqkv_blk[blk, 8 + 2 * hh + d, :]
            wv96[64, 3 * pp + d] = bqkv_blk[blk, 8 + 2 * hh + d]
        wv96[64, 3 * pp + 2] = 1.0

    # cross Q/K lhsT tiles: head h=4j+s at columns [32s, 32s+8)
    wcq = np.zeros((2, 65, 128), np.float32)
    wck = np.zeros((2, 65, 128), np.float32)
    for h in range(8):
        j, s = h // 4, h % 4
        wcq[j, :64, 32 * s:32 * s + 8] = Wqkv_c[8 * h:8 * h + 8, :].T
        wcq[j, 64, 32 * s:32 * s + 8] = bqkv_c[8 * h:8 * h + 8]
        wck[j, :64, 32 * s:32 * s + 8] = Wqkv_c[64 + 8 * h:64 + 8 * h + 8, :].T
        wck[j, 64, 32 * s:32 * s + 8] = bqkv_c[64 + 8 * h:64 + 8 * h + 8]

    wcv72 = np.zeros((65, 72), np.float32)
    for h in range(8):
        wcv72[:64, 9 * h:9 * h + 8] = Wqkv_c[128 + 8 * h:128 + 8 * h + 8, :].T
        wcv72[64, 9 * h:9 * h + 8] = bqkv_c[128 + 8 * h:128 + 8 * h + 8]
        wcv72[64, 9 * h + 8] = 1.0

    wobig = np.zeros((65, 64), np.float32)
    for blk in range(NBLK):
        wobig[4 * blk:4 * blk + 4, 4 * blk:4 * blk + 4] = Wo_blk[blk].T
        wobig[64, 4 * blk:4 * blk + 4] = bo_blk[blk]

    woc = np.zeros((65, 64), np.float32)
    woc[:64, :] = Wo_c.T
    woc[64, :] = bo_c
    w1 = np.zeros((65, 256), np.float32)
    w1[:64, :] = W1.T
    w1[64, :] = b1
    w2 = np.ascontiguousarray(W2.T).reshape(2, 128, 64)
    lnp = np.stack([np.asarray(inp['g1'], np.float32), np.asarray(inp['be1'], np.float32),
                    np.asarray(inp['g2'], np.float32), np.asarray(inp['be2'], np.float32)])
    return dict(wq=wq, wk=wk, wv96=wv96, wcq=wcq, wck=wck, wcv72=wcv72,
                wobig=wobig, woc=woc, w1=w1, w2=w2,
                b2=b2.reshape(64, 1).copy(),
                lnp=np.ascontiguousarray(lnp),
                alpha=np.asarray(inp['sens_alpha'], np.float32).reshape(1, 16).copy(),
                semb=np.asarray(inp['sens_emb'], np.float32),
                ident=np.eye(128, dtype=np.float32),
                e4=np.stack([np.eye(4, dtype=np.float32)[t].reshape(4, 1)
                             * np.ones((1, 128), np.float32) for t in range(4)]))


_NC_CACHE = None

# ---------------------------------------------------------------------------
# Fast dispatch layer.
#
# The axon tunnel makes every byte and every round trip expensive (measured:
# ~30-70 ms base latency per H2D transfer at ~38 MB/s, ~66 ms per execute
# round trip, D2H effectively free).  The baseline re-traced/re-jitted the
# program and re-uploaded all ~28 MB of (mostly constant) per-core inputs on
# every call.  Instead we keep
#   - one AOT-compiled fast-dispatch executable (bass_effect suppressed so
#     pjit's C++ fast path runs), built once per process;
#   - every kernel input device-resident, keyed by CRC of the raw host
#     arrays (params and M/token_ids tracked separately so a data-only
#     change re-uploads ~4 MB, not 28 MB);
#   - one device-resident dummy buffer for the NEFF's "out" operand.  The
#     program writes every element of out before anything reads it, so the
#     operand's contents never matter and it is never re-uploaded.
# Warm calls with unchanged inputs transfer nothing host->device.
# ---------------------------------------------------------------------------

_PARAM_DEPS = ('Wqkv_blk', 'bqkv_blk', 'Wo_blk', 'bo_blk', 'Wqkv_c', 'bqkv_c',
               'Wo_c', 'bo_c', 'W1', 'b1', 'W2', 'b2', 'g1', 'be1', 'g2',
               'be2', 'sens_emb', 'sens_alpha')
_DATA_NAMES = ('xt_aug', 'xtq_aug', 'x_tok', 'tokidx')


def _crc(a):
    import zlib
    a = np.ascontiguousarray(a)
    return zlib.crc32(memoryview(a.view(np.uint8).reshape(-1)))


def _build_in_maps(inputs, params):
    M = np.asarray(inputs['M'], np.float32)
    tok = np.asarray(inputs['token_ids']).astype(np.int64)
    blocks = _to_blocks(M)                       # [B, T, 64]
    in_maps = []
    for c in range(8):
        b, h = c // 2, c % 2
        xt = np.concatenate([blocks[b].T, np.ones((1, T), np.float32)], axis=0)
        m = dict(params)
        m['xt_aug'] = np.ascontiguousarray(xt)
        m['xtq_aug'] = np.ascontiguousarray(xt[:, h * TQ:(h + 1) * TQ])
        m['x_tok'] = np.ascontiguousarray(blocks[b, h * TQ:(h + 1) * TQ, :])
        ids = tok[b, h * TQ:(h + 1) * TQ].astype(np.int32)
        m['tokidx'] = np.ascontiguousarray(ids.reshape(4, P).T)  # [p,c]: t=c*128+p
        in_maps.append(m)
    return in_maps


class _Ctx:
    def __init__(self):
        self.nc = None
        self.compiled = None
        self.in_names = None
        self.out_names = None
        self.sharding = None
        self.dev = {}            # name -> device array (committed, sharded)
        self.param_key = None
        self.data_key = None
        self.out_operand = None  # device-resident dummy for the out buffer


_CTX = None


def _get_ctx():
    global _CTX
    if _CTX is None:
        _CTX = _Ctx()
        import jax
        from jax.sharding import Mesh, PartitionSpec, NamedSharding
        _CTX.mesh = Mesh(np.asarray(jax.devices()[:8]), ("core",))
        _CTX.sharding = NamedSharding(_CTX.mesh, PartitionSpec("core"))
    return _CTX


def _dev_put(ctx, name, host_arrs):
    """Concat per-core arrays for `name` and place sharded on the 8 cores."""
    import jax
    glob = np.concatenate(host_arrs, axis=0)
    ctx.dev[name] = jax.device_put(glob, ctx.sharding)


def _build_compiled(ctx, nc):
    import jax
    from jax.sharding import Mesh, PartitionSpec, NamedSharding
    try:
        from jax.experimental.shard_map import shard_map
    except ImportError:
        from jax import shard_map
    from concourse import bass2jax

    bass2jax.install_neuronx_cc_hook()
    partition_name = (nc.partition_id_tensor.name
                      if nc.partition_id_tensor else None)
    in_names, out_names, out_avals = [], [], []
    for alloc in nc.m.functions[0].allocations:
        if not isinstance(alloc, mybir.MemoryLocationSet):
            continue
        name = alloc.memorylocations[0].name
        if alloc.kind == "ExternalInput":
            if name != partition_name:
                in_names.append(name)
        elif alloc.kind == "ExternalOutput":
            out_names.append(name)
            out_avals.append(jax.core.ShapedArray(
                tuple(alloc.tensor_shape), mybir.dt.np(alloc.dtype)))
    in_names_all = list(in_names) + list(out_names)
    if partition_name is not None:
        in_names_all.append(partition_name)

    def _body(*args):
        operands = list(args)
        if partition_name is not None:
            operands.append(bass2jax.partition_id_tensor())
        return tuple(bass2jax._bass_exec_p.bind(
            *operands,
            out_avals=tuple(out_avals),
            in_names=tuple(in_names_all),
            out_names=tuple(out_names),
            lowering_input_output_aliases=(),
            sim_require_finite=True,
            sim_require_nnan=True,
            nc=nc,
        ))

    n_cores = 8
    mesh = ctx.mesh
    n_args = len(in_names) + len(out_names)

    ctx.in_names = in_names
    ctx.out_names = out_names
    ctx.out_avals = out_avals

    def _compile():
        jitted = jax.jit(
            shard_map(_body, mesh=mesh,
                      in_specs=(PartitionSpec("core"),) * n_args,
                      out_specs=(PartitionSpec("core"),) * len(out_names),
                      check_rep=False),
            keep_unused=True)
        arg_structs = []
        for name in in_names:
            a = ctx.dev[name]
            arg_structs.append(jax.ShapeDtypeStruct(a.shape, a.dtype,
                                                    sharding=a.sharding))
        for av in out_avals:
            arg_structs.append(jax.ShapeDtypeStruct(
                (n_cores * av.shape[0],) + tuple(av.shape[1:]), av.dtype,
                sharding=ctx.sharding))
        return jitted.lower(*arg_structs).compile()

    try:
        ctx.compiled = bass2jax.fast_dispatch_compile(_compile)
    except Exception:
        ctx.compiled = _compile()

    # dummy out operand (contents never read; program writes all of out)
    av = out_avals[0]
    ctx.out_operand = jax.device_put(
        np.zeros((n_cores * av.shape[0],) + tuple(av.shape[1:]), av.dtype),
        ctx.sharding)


def kernel(**inputs):
    global _NC_CACHE
    import jax

    ctx = _get_ctx()
    param_key = tuple(_crc(np.asarray(inputs[k])) for k in _PARAM_DEPS)
    data_key = (_crc(np.asarray(inputs['M'])),
                _crc(np.asarray(inputs['token_ids'])))

    params_stale = param_key != ctx.param_key
    data_stale = params_stale or data_key != ctx.data_key
    if data_stale:
        if params_stale:
            ctx._last_params = _host_params(inputs)
        in_maps = _build_in_maps(inputs, ctx._last_params)
        if ctx.compiled is None:
            if _NC_CACHE is None:
                _NC_CACHE = build_program()
            ctx.nc = _NC_CACHE
            # stage every input on device first (compile needs shardings)
            for name in in_maps[0]:
                _dev_put(ctx, name, [m[name] for m in in_maps])
            _build_compiled(ctx, ctx.nc)
        else:
            stale = in_maps[0].keys() if params_stale else _DATA_NAMES
            for name in stale:
                _dev_put(ctx, name, [m[name] for m in in_maps])
        ctx.param_key = param_key
        ctx.data_key = data_key

    args = [ctx.dev[name] for name in ctx.in_names]
    args.append(ctx.out_operand)
    outs = ctx.compiled(*args)
    res = np.asarray(outs[0]).reshape(8, TQ, 64)

    out_blocks = np.empty((B, T, 64), np.float32)
    for c in range(8):
        b, h = c // 2, c % 2
        out_blocks[b, h * TQ:(h + 1) * TQ, :] = res[c]
    return _from_blocks(out_blocks.reshape(B, T, 16, 4))


if __name__ == '__main__':
    import reference as ref
    import jax
    inp = {k: np.asarray(jax.device_get(v)) for k, v in ref.setup_inputs().items()}
    got = kernel(**inp)
    exp = np.asarray(jax.device_get(ref.reference(**{k: v for k, v in inp.items()})))
    err = np.abs(got - exp)
    print(f"max abs {err.max():.3e}  scale-rel {err.max()/np.abs(exp).max():.3e}")

